# revision 10
# baseline (speedup 1.0000x reference)
"""MoE feed-forward (top-1 routing, capacity 640, swiglu experts) on 8 trn2 cores.

Strategy (expert-parallel, as per sharding hint):
  * Host: router matmul/softmax/argmax + capacity-slot assignment (index
    plumbing, ~0.1% of FLOPs), gathers tokens per expert, shards 2 experts
    per core.
  * Device (Bass/Tile, per core): grouped GEMM  h = x @ W1  -> swiglu ->
    y = g @ W2, weighted by combine gates.  Matmuls in bf16 with fp32
    accumulate (bf16 keeps LDWEIGHTS off the critical path).  GEMM1
    computes hT [feat, tok] so GEMM2 needs no on-chip transpose.
  * Host: scatter weighted expert outputs back to token order; dense
    fallback FFN applied only to dropped tokens (none for typical loads).
"""

import os
import sys

import numpy as np


def _ensure_concourse():
    try:
        import concourse.bass  # noqa: F401
    except Exception:
        for p in ("/opt/trn_rl_repo", "/root/.axon_site/_ro/trn_rl_repo"):
            if os.path.isdir(p) and p not in sys.path:
                sys.path.insert(0, p)
        import concourse.bass  # noqa: F401


# Problem constants (hardcoded per the task contract).
B, S, D, H, E = 4, 2048, 768, 3072, 16
N = B * S
C = 640  # capacity per expert (ceil(1.25 * N / E))
FALLBACK_W = 1.0
NCORES = 8
EL = E // NCORES  # experts per core = 2
KD = D // 128  # 6 k-tiles for GEMM1 contraction
FB = (2 * H) // 128  # 48 feature blocks of GEMM1 output
FP = FB // 2  # 24 swiglu pairs == k-tiles of GEMM2 contraction
KH = H // 128  # 24
TOK = 320  # token half-tile (2 x 320 = 640); >=256 keeps fp32r at full rate
MT = C // 128  # 5 token m-tiles for GEMM2
DH = 384  # output d half-tile (2 x 384 = 768)

_NC = None  # cached Bass program
_WCACHE = {}  # weight reorder cache
LAST = None  # BassKernelResults of the most recent run (for profiling)


def _build_nc():
    """Per-core Bass program: 2 experts x (GEMM1 + swiglu + GEMM2 + gate)."""
    import concourse.bacc as bacc
    import concourse.bass as bass  # noqa: F401
    import concourse.mybir as mybir
    import concourse.tile as tile
    from contextlib import ExitStack

    f32 = mybir.dt.float32
    f32r = mybir.dt.float32r
    bf16 = mybir.dt.bfloat16
    AF = mybir.ActivationFunctionType
    ALU = mybir.AluOpType

    nc = bacc.Bacc("TRN2", target_bir_lowering=False)
    # Host-side layouts are pre-tiled so every DMA is 2D [128, contiguous].
    xt = nc.dram_tensor("xt", [EL, 128, KD * C], bf16, kind="ExternalInput")
    w1r = nc.dram_tensor("w1r", [EL, FB, 128, KD * 128], bf16, kind="ExternalInput")
    w2t = nc.dram_tensor("w2t", [EL, 128, KH * D], bf16, kind="ExternalInput")
    b1t = nc.dram_tensor("b1t", [EL, 128, FB], f32, kind="ExternalInput")
    wce = nc.dram_tensor("wce", [EL, 128, MT], f32, kind="ExternalInput")
    y = nc.dram_tensor("y", [EL, C, D], f32, kind="ExternalOutput")

    with tile.TileContext(nc) as tc, ExitStack() as ctx:
        xp = ctx.enter_context(tc.tile_pool(name="xp", bufs=2))
        w2p = ctx.enter_context(tc.tile_pool(name="w2p", bufs=2))
        gp = ctx.enter_context(tc.tile_pool(name="gp", bufs=2))
        w1p = ctx.enter_context(tc.tile_pool(name="w1p", bufs=4))
        sap = ctx.enter_context(tc.tile_pool(name="sap", bufs=3))
        cst = ctx.enter_context(tc.tile_pool(name="cst", bufs=2))
        yp = ctx.enter_context(tc.tile_pool(name="yp", bufs=4))
        p1 = ctx.enter_context(tc.tile_pool(name="p1", bufs=3, space="PSUM"))
        p2 = ctx.enter_context(tc.tile_pool(name="p2", bufs=2, space="PSUM"))

        for e in range(EL):
            xsb = xp.tile([128, KD * C], bf16, tag="x")
            nc.gpsimd.dma_start(xsb[:], xt[e, :, :])
            b1sb = cst.tile([128, FB], f32, tag="b1")
            nc.gpsimd.dma_start(b1sb[:], b1t[e, :, :])
            wcsb = cst.tile([128, MT], f32, tag="wc")
            nc.gpsimd.dma_start(wcsb[:], wce[e, :, :])

            gt = gp.tile([128, KH * C], bf16, tag="g")

            # GEMM1 + swiglu: hT tiles [feat 128, tok 320]
            for fp in range(FP):
                w1a = w1p.tile([128, KD * 128], bf16, tag="w1a")
                nc.gpsimd.dma_start(w1a[:], w1r[e, fp, :, :])
                w1b = w1p.tile([128, KD * 128], bf16, tag="w1b")
                nc.gpsimd.dma_start(w1b[:], w1r[e, FP + fp, :, :])
                for t in range(2):
                    pa = p1.tile([128, TOK], f32, tag="pa")
                    pb = p1.tile([128, TOK], f32, tag="pb")
                    for k in range(KD):
                        nc.tensor.matmul(
                            pa[:],
                            lhsT=w1a[:, k * 128 : (k + 1) * 128],
                            rhs=xsb[:, k * C + t * TOK : k * C + (t + 1) * TOK],
                            start=(k == 0),
                            stop=(k == KD - 1),
                        )
                    for k in range(KD):
                        nc.tensor.matmul(
                            pb[:],
                            lhsT=w1b[:, k * 128 : (k + 1) * 128],
                            rhs=xsb[:, k * C + t * TOK : k * C + (t + 1) * TOK],
                            start=(k == 0),
                            stop=(k == KD - 1),
                        )
                    sa = sap.tile([128, TOK], f32, tag="sa")
                    # silu(a + b1_a)
                    nc.scalar.activation(
                        sa[:], pa[:], AF.Silu, bias=b1sb[:, fp : fp + 1], scale=1.0
                    )
                    # g = (b + b1_b) * silu(...)
                    nc.vector.scalar_tensor_tensor(
                        out=gt[:, fp * C + t * TOK : fp * C + (t + 1) * TOK],
                        in0=pb[:],
                        scalar=b1sb[:, FP + fp : FP + fp + 1],
                        in1=sa[:],
                        op0=ALU.add,
                        op1=ALU.mult,
                    )

            # GEMM2: y[tok 128, d 384] = sum_k g[tok, h_k] @ W2[h_k, d]
            w2sb = w2p.tile([128, KH * D], bf16, tag="w2")
            nc.gpsimd.dma_start(w2sb[:], w2t[e, :, :])
            for m in range(MT):
                for dh in range(2):
                    pt = p2.tile([128, DH], f32, tag="p2")
                    for k in range(KH):
                        nc.tensor.matmul(
                            pt[:],
                            lhsT=gt[:, k * C + m * 128 : k * C + m * 128 + 128],
                            rhs=w2sb[:, k * D + dh * DH : k * D + (dh + 1) * DH],
                            start=(k == 0),
                            stop=(k == KH - 1),
                        )
                    ysb = yp.tile([128, DH], f32, tag="y")
                    # weighted combine: y *= gate (per-token scalar); b2 is
                    # handled host-side (it is all zeros for this problem).
                    nc.scalar.activation(
                        ysb[:], pt[:], AF.Copy, bias=0.0, scale=wcsb[:, m : m + 1]
                    )
                    nc.gpsimd.dma_start(
                        y[e, m * 128 : (m + 1) * 128, dh * DH : (dh + 1) * DH], ysb[:]
                    )
    nc.compile()
    return nc


def _get_nc():
    global _NC
    if _NC is None:
        _NC = _build_nc()
    return _NC


def _reorder_weights(W1, W2, b1):
    key = (W1.__array_interface__["data"][0], W2.__array_interface__["data"][0])
    hit = _WCACHE.get(key)
    if hit is not None:
        return hit
    W1 = np.ascontiguousarray(W1, dtype=np.float32)
    W2 = np.ascontiguousarray(W2, dtype=np.float32)
    b1 = np.ascontiguousarray(b1, dtype=np.float32)
    # W1 [E, D, 2H] -> [E, FB, 128p(d within k), KD*128(f)]
    import ml_dtypes

    w1r = np.ascontiguousarray(
        W1.reshape(E, KD, 128, FB, 128)
        .transpose(0, 3, 2, 1, 4)
        .reshape(E, FB, 128, KD * 128)
        .astype(ml_dtypes.bfloat16)
    )
    # W2 [E, H, D] -> [E, 128p(h within k), KH*D]
    w2t = np.ascontiguousarray(
        W2.reshape(E, KH, 128, D)
        .transpose(0, 2, 1, 3)
        .reshape(E, 128, KH * D)
        .astype(ml_dtypes.bfloat16)
    )
    # b1 [E, 2H] -> [E, 128, FB]
    b1t = b1.reshape(E, FB, 128).transpose(0, 2, 1).copy()
    out = (w1r, w2t, b1t)
    _WCACHE.clear()
    _WCACHE[key] = out
    return out


def _route(x_flat, Wr):
    logits = x_flat @ np.ascontiguousarray(Wr, dtype=np.float32)  # [N, E]
    lmax = logits.max(axis=-1, keepdims=True)
    p = np.exp(logits - lmax)
    gates = p / p.sum(axis=-1, keepdims=True)
    expert = np.argmax(gates, axis=-1)
    # slot = occurrence index of each token within its expert's queue
    order = np.argsort(expert, kind="stable")
    sorted_e = expert[order]
    starts = np.searchsorted(sorted_e, np.arange(E))
    within = np.arange(N) - starts[sorted_e]
    slot = np.empty(N, np.int64)
    slot[order] = within
    kept = slot < C
    top_idx = np.zeros((C, E), np.int32)
    valid = np.zeros((C, E), np.float32)
    tok = np.arange(N, dtype=np.int32)
    top_idx[slot[kept], expert[kept]] = tok[kept]
    valid[slot[kept], expert[kept]] = 1.0
    w_ce = gates[top_idx, np.arange(E)[None, :]].astype(np.float32) * valid  # [C, E]
    return gates, expert, kept, top_idx, valid, w_ce


def kernel(x, Wr, W1, b1, W2, b2, W1f, b1f, W2f, b2f, _trace=False):
    global LAST
    _ensure_concourse()
    from concourse.bass_utils import run_bass_kernel_spmd

    x_flat = np.ascontiguousarray(np.asarray(x).reshape(N, D), dtype=np.float32)
    gates, expert, kept, top_idx, valid, w_ce = _route(x_flat, np.asarray(Wr))
    w1r, w2t, b1t = _reorder_weights(np.asarray(W1), np.asarray(W2), np.asarray(b1))

    # Gather tokens per expert: [E, C, D]; invalid slots carry garbage rows,
    # they are zeroed by the gate weight (w_ce == 0 there).
    x_g = x_flat[top_idx.T]  # [E, C, D]
    # xT tiles: [E, 128p(d within k), KD*C]
    import ml_dtypes

    xT = x_g.reshape(E, C, KD, 128).transpose(0, 3, 2, 1).reshape(E, 128, KD * C)
    xT = np.ascontiguousarray(xT, dtype=ml_dtypes.bfloat16)
    # combine weights per expert: [E, 128, MT]
    wct = np.ascontiguousarray(w_ce.T.reshape(E, MT, 128).transpose(0, 2, 1))

    nc = _get_nc()
    in_maps = []
    for c in range(NCORES):
        sl = slice(c * EL, (c + 1) * EL)
        in_maps.append(
            {
                "xt": np.ascontiguousarray(xT[sl]),
                "w1r": np.ascontiguousarray(w1r[sl]),
                "w2t": np.ascontiguousarray(w2t[sl]),
                "b1t": np.ascontiguousarray(b1t[sl]),
                "wce": np.ascontiguousarray(wct[sl]),
            }
        )
    res = run_bass_kernel_spmd(nc, in_maps, list(range(NCORES)), trace=_trace)
    LAST = res

    # Combine: scatter weighted expert outputs back to token order.
    y_flat = np.zeros((N, D), np.float32)
    y_w = np.concatenate([r["y"] for r in res.results], axis=0)  # [E, C, D]
    mask = valid.astype(bool)  # [C, E]
    y_flat[top_idx[mask]] = y_w.transpose(1, 0, 2)[mask]
    if np.any(b2):
        eb = np.nonzero(mask)[1]
        y_flat[top_idx[mask]] += w_ce[mask][:, None] * np.asarray(b2)[eb]

    # Dense fallback for fully-dropped tokens (rare; none at typical loads).
    dropped = ~kept
    if np.any(dropped):
        xd = x_flat[dropped]
        hf = xd @ np.asarray(W1f) + np.asarray(b1f)
        gf = (hf[:, :H] / (1.0 + np.exp(-hf[:, :H]))) * hf[:, H:]
        y_flat[dropped] += FALLBACK_W * (gf @ np.asarray(W2f) + np.asarray(b2f))

    return y_flat.reshape(B, S, D)


# revision 11
# speedup vs baseline: 1.0753x; 1.0753x over previous
"""MoE feed-forward (top-1 routing, capacity 640, swiglu experts) on 8 trn2 cores.

Strategy (expert-parallel, per the sharding hint):
  * Host: router matmul/softmax/argmax + capacity-slot assignment (index
    plumbing, ~0.1% of FLOPs), gathers tokens per expert, pairs a heavy
    expert with a light one per core (greedy balance), 2 experts per core.
  * Device (Bass/Tile, per core): grouped GEMM  h = x @ W1  -> swiglu ->
    y = g @ W2, weighted by combine gates.  Matmuls in bf16 with fp32
    accumulate (bf16 keeps LDWEIGHTS off the critical path).  GEMM1
    computes hT [feat, tok] so GEMM2 needs no on-chip transpose.  Token
    tiles are sized to the actual routed token counts (rounded up to 128)
    instead of the worst-case capacity; the program is cached per
    (mt0, mt1) m-tile profile.
  * Host: scatter weighted expert outputs back to token order; dense
    fallback FFN applied only to dropped tokens (none at typical loads).
"""

import os
import sys

import numpy as np


def _ensure_concourse():
    try:
        import concourse.bass  # noqa: F401
    except Exception:
        for p in ("/opt/trn_rl_repo", "/root/.axon_site/_ro/trn_rl_repo"):
            if os.path.isdir(p) and p not in sys.path:
                sys.path.insert(0, p)
        import concourse.bass  # noqa: F401


# Problem constants (hardcoded per the task contract).
B, S, D, H, E = 4, 2048, 768, 3072, 16
N = B * S
C = 640  # capacity per expert (ceil(1.25 * N / E))
FALLBACK_W = 1.0
NCORES = 8
EL = E // NCORES  # experts per core = 2
KD = D // 128  # 6 k-tiles for GEMM1 contraction
FB = (2 * H) // 128  # 48 feature blocks of GEMM1 output
FP = FB // 2  # 24 swiglu pairs == k-tiles of GEMM2 contraction
KH = H // 128  # 24
MT = C // 128  # max token m-tiles per expert
DH = 384  # output d half-tile (2 x 384 = 768)

_NC_CACHE = {}  # (mt0, mt1) -> compiled Bass program
_WCACHE = {}  # weight reorder cache
LAST = None  # BassKernelResults of the most recent run (for profiling)


def _tok_tiles(pad):
    """Split a padded token count into moving-operand tiles (<=512)."""
    out, off = [], 0
    while pad - off > 512:
        out.append((off, 512))
        off += 512
    out.append((off, pad - off))
    return out


def _build_nc(mts):
    """Per-core Bass program: 2 expert slots with mts[s] token m-tiles each."""
    import concourse.bacc as bacc
    import concourse.mybir as mybir
    import concourse.tile as tile
    from contextlib import ExitStack

    f32 = mybir.dt.float32
    bf16 = mybir.dt.bfloat16
    AF = mybir.ActivationFunctionType
    ALU = mybir.AluOpType

    pads = [m * 128 for m in mts]
    tot = sum(pads)

    nc = bacc.Bacc("TRN2", target_bir_lowering=False)
    # Host-side layouts are pre-tiled so every DMA is 2D [128, contiguous].
    xt = nc.dram_tensor("xt", [128, KD * tot], bf16, kind="ExternalInput")
    w1r = nc.dram_tensor("w1r", [EL, FB, 128, KD * 128], bf16, kind="ExternalInput")
    w2t = nc.dram_tensor("w2t", [EL, 128, KH * D], bf16, kind="ExternalInput")
    b1t = nc.dram_tensor("b1t", [EL, 128, FB], f32, kind="ExternalInput")
    wce = nc.dram_tensor("wce", [EL, 128, MT], f32, kind="ExternalInput")
    y = nc.dram_tensor("y", [tot, D], f32, kind="ExternalOutput")

    with tile.TileContext(nc) as tc, ExitStack() as ctx:
        xp = ctx.enter_context(tc.tile_pool(name="xp", bufs=2))
        w2p = ctx.enter_context(tc.tile_pool(name="w2p", bufs=2))
        gp = ctx.enter_context(tc.tile_pool(name="gp", bufs=2))
        w1p = ctx.enter_context(tc.tile_pool(name="w1p", bufs=4))
        sap = ctx.enter_context(tc.tile_pool(name="sap", bufs=3))
        cst = ctx.enter_context(tc.tile_pool(name="cst", bufs=2))
        yp = ctx.enter_context(tc.tile_pool(name="yp", bufs=4))
        p1 = ctx.enter_context(tc.tile_pool(name="p1", bufs=3, space="PSUM"))
        p2 = ctx.enter_context(tc.tile_pool(name="p2", bufs=2, space="PSUM"))

        for e in range(EL):
            pad = pads[e]
            xoff = KD * pads[0] if e else 0
            tiles = _tok_tiles(pad)
            xsb = xp.tile([128, KD * pad], bf16, tag="x")
            # per-k chunks so the first matmul doesn't wait for the full load
            for k in range(KD):
                nc.gpsimd.dma_start(
                    xsb[:, k * pad : (k + 1) * pad],
                    xt[:, xoff + k * pad : xoff + (k + 1) * pad],
                )
            b1sb = cst.tile([128, FB], f32, tag="b1")
            nc.gpsimd.dma_start(b1sb[:], b1t[e, :, :])
            wcsb = cst.tile([128, MT], f32, tag="wc")
            nc.gpsimd.dma_start(wcsb[:], wce[e, :, :])

            gt = gp.tile([128, KH * pad], bf16, tag="g")

            # GEMM1 + swiglu: hT tiles [feat 128, tok <=512]
            for fp in range(FP):
                w1a = w1p.tile([128, KD * 128], bf16, tag="w1a")
                nc.gpsimd.dma_start(w1a[:], w1r[e, fp, :, :])
                w1b = w1p.tile([128, KD * 128], bf16, tag="w1b")
                nc.gpsimd.dma_start(w1b[:], w1r[e, FP + fp, :, :])
                for toff, tn in tiles:
                    pa = p1.tile([128, tn], f32, tag="pa")
                    pb = p1.tile([128, tn], f32, tag="pb")
                    for k in range(KD):
                        nc.tensor.matmul(
                            pa[:],
                            lhsT=w1a[:, k * 128 : (k + 1) * 128],
                            rhs=xsb[:, k * pad + toff : k * pad + toff + tn],
                            start=(k == 0),
                            stop=(k == KD - 1),
                        )
                    for k in range(KD):
                        nc.tensor.matmul(
                            pb[:],
                            lhsT=w1b[:, k * 128 : (k + 1) * 128],
                            rhs=xsb[:, k * pad + toff : k * pad + toff + tn],
                            start=(k == 0),
                            stop=(k == KD - 1),
                        )
                    sa = sap.tile([128, tn], f32, tag="sa")
                    # silu(a + b1_a)
                    nc.scalar.activation(
                        sa[:], pa[:], AF.Silu, bias=b1sb[:, fp : fp + 1], scale=1.0
                    )
                    # g = (b + b1_b) * silu(...)
                    nc.vector.scalar_tensor_tensor(
                        out=gt[:, fp * pad + toff : fp * pad + toff + tn],
                        in0=pb[:],
                        scalar=b1sb[:, FP + fp : FP + fp + 1],
                        in1=sa[:],
                        op0=ALU.add,
                        op1=ALU.mult,
                    )

            # GEMM2: y[tok 128, d 384] = sum_k g[tok, h_k] @ W2[h_k, d]
            w2sb = w2p.tile([128, KH * D], bf16, tag="w2")
            nc.gpsimd.dma_start(w2sb[:], w2t[e, :, :])
            yoff = pads[0] if e else 0
            for m in range(mts[e]):
                for dh in range(2):
                    pt = p2.tile([128, DH], f32, tag="p2")
                    for k in range(KH):
                        nc.tensor.matmul(
                            pt[:],
                            lhsT=gt[:, k * pad + m * 128 : k * pad + m * 128 + 128],
                            rhs=w2sb[:, k * D + dh * DH : k * D + (dh + 1) * DH],
                            start=(k == 0),
                            stop=(k == KH - 1),
                        )
                    ysb = yp.tile([128, DH], f32, tag="y")
                    # weighted combine: y *= gate (per-token scalar); b2 is
                    # handled host-side (it is all zeros for this problem).
                    nc.scalar.activation(
                        ysb[:], pt[:], AF.Copy, bias=0.0, scale=wcsb[:, m : m + 1]
                    )
                    nc.gpsimd.dma_start(
                        y[
                            yoff + m * 128 : yoff + (m + 1) * 128,
                            dh * DH : (dh + 1) * DH,
                        ],
                        ysb[:],
                    )
    nc.compile()
    return nc


def _get_nc(mts):
    nc = _NC_CACHE.get(mts)
    if nc is None:
        nc = _NC_CACHE[mts] = _build_nc(mts)
    return nc


def _reorder_weights(W1, W2, b1):
    key = (W1.__array_interface__["data"][0], W2.__array_interface__["data"][0])
    hit = _WCACHE.get(key)
    if hit is not None:
        return hit
    import ml_dtypes

    W1 = np.ascontiguousarray(W1, dtype=np.float32)
    W2 = np.ascontiguousarray(W2, dtype=np.float32)
    b1 = np.ascontiguousarray(b1, dtype=np.float32)
    # W1 [E, D, 2H] -> [E, FB, 128p(d within k), KD*128(f)]
    w1r = np.ascontiguousarray(
        W1.reshape(E, KD, 128, FB, 128)
        .transpose(0, 3, 2, 1, 4)
        .reshape(E, FB, 128, KD * 128)
        .astype(ml_dtypes.bfloat16)
    )
    # W2 [E, H, D] -> [E, 128p(h within k), KH*D]
    w2t = np.ascontiguousarray(
        W2.reshape(E, KH, 128, D)
        .transpose(0, 2, 1, 3)
        .reshape(E, 128, KH * D)
        .astype(ml_dtypes.bfloat16)
    )
    # b1 [E, 2H] -> [E, 128, FB]
    b1t = np.ascontiguousarray(b1.reshape(E, FB, 128).transpose(0, 2, 1))
    out = (w1r, w2t, b1t)
    _WCACHE.clear()
    _WCACHE[key] = out
    return out


def _route(x_flat, Wr):
    logits = x_flat @ np.ascontiguousarray(Wr, dtype=np.float32)  # [N, E]
    lmax = logits.max(axis=-1, keepdims=True)
    p = np.exp(logits - lmax)
    gates = p / p.sum(axis=-1, keepdims=True)
    expert = np.argmax(gates, axis=-1)
    # slot = occurrence index of each token within its expert's queue
    order = np.argsort(expert, kind="stable")
    sorted_e = expert[order]
    starts = np.searchsorted(sorted_e, np.arange(E))
    within = np.arange(N) - starts[sorted_e]
    slot = np.empty(N, np.int64)
    slot[order] = within
    kept = slot < C
    top_idx = np.zeros((C, E), np.int32)
    valid = np.zeros((C, E), np.float32)
    tok = np.arange(N, dtype=np.int32)
    top_idx[slot[kept], expert[kept]] = tok[kept]
    valid[slot[kept], expert[kept]] = 1.0
    w_ce = gates[top_idx, np.arange(E)[None, :]].astype(np.float32) * valid  # [C, E]
    n_kept = np.minimum(np.bincount(expert, minlength=E), C)  # [E]
    return gates, expert, kept, top_idx, valid, w_ce, n_kept


def kernel(x, Wr, W1, b1, W2, b2, W1f, b1f, W2f, b2f, _trace=False):
    global LAST
    _ensure_concourse()
    import ml_dtypes
    from concourse.bass_utils import run_bass_kernel_spmd

    x_flat = np.ascontiguousarray(np.asarray(x).reshape(N, D), dtype=np.float32)
    gates, expert, kept, top_idx, valid, w_ce, n_kept = _route(x_flat, np.asarray(Wr))
    w1r, w2t, b1t = _reorder_weights(np.asarray(W1), np.asarray(W2), np.asarray(b1))

    # Pair heavy experts with light ones (greedy balance); slot 0 = heavy.
    order = np.argsort(-n_kept, kind="stable")
    assign = [(int(order[i]), int(order[E - 1 - i])) for i in range(NCORES)]
    mt_of = [max(1, int(-(-n // 128))) for n in n_kept]
    mts = (
        max(mt_of[a] for a, _ in assign),
        max(mt_of[b] for _, b in assign),
    )
    pads = [m * 128 for m in mts]

    nc = _get_nc(mts)
    in_maps = []
    for c in range(NCORES):
        exps = assign[c]
        # gather + transpose tokens for each slot: [128, KD * pad]
        xparts = []
        for s, e in enumerate(exps):
            ids = top_idx[: n_kept[e], e]
            xg = np.zeros((pads[s], D), np.float32)
            xg[: len(ids)] = x_flat[ids]
            xparts.append(
                xg.reshape(pads[s], KD, 128)
                .transpose(2, 1, 0)
                .reshape(128, KD * pads[s])
            )
        xt_c = np.ascontiguousarray(
            np.concatenate(xparts, axis=1), dtype=ml_dtypes.bfloat16
        )
        wct = np.zeros((EL, 128, MT), np.float32)
        for s, e in enumerate(exps):
            w = np.zeros(pads[s], np.float32)
            w[: n_kept[e]] = w_ce[: n_kept[e], e]
            wct[s, :, : mts[s]] = w.reshape(mts[s], 128).T
        el = list(exps)
        in_maps.append(
            {
                "xt": xt_c,
                "w1r": np.ascontiguousarray(w1r[el]),
                "w2t": np.ascontiguousarray(w2t[el]),
                "b1t": np.ascontiguousarray(b1t[el]),
                "wce": wct,
            }
        )
    res = run_bass_kernel_spmd(nc, in_maps, list(range(NCORES)), trace=_trace)
    LAST = res

    # Combine: scatter weighted expert outputs back to token order.
    y_flat = np.zeros((N, D), np.float32)
    b2 = np.asarray(b2)
    add_b2 = bool(np.any(b2))
    for c in range(NCORES):
        yc = res.results[c]["y"]
        for s, e in enumerate(assign[c]):
            n = int(n_kept[e])
            ids = top_idx[:n, e]
            off = pads[0] if s else 0
            y_flat[ids] = yc[off : off + n]
            if add_b2:
                y_flat[ids] += w_ce[:n, e][:, None] * b2[e]

    # Dense fallback for fully-dropped tokens (rare; none at typical loads).
    dropped = ~kept
    if np.any(dropped):
        xd = x_flat[dropped]
        hf = xd @ np.asarray(W1f) + np.asarray(b1f)
        gf = (hf[:, :H] / (1.0 + np.exp(-hf[:, :H]))) * hf[:, H:]
        y_flat[dropped] += FALLBACK_W * (gf @ np.asarray(W2f) + np.asarray(b2f))

    return y_flat.reshape(B, S, D)


# revision 12
# speedup vs baseline: 1.1142x; 1.0361x over previous
"""MoE feed-forward (top-1 routing, capacity 640, swiglu experts) on 8 trn2 cores.

Strategy (expert-parallel, per the sharding hint):
  * Host: router matmul/softmax/argmax + capacity-slot assignment (index
    plumbing, ~0.1% of FLOPs), gathers tokens per expert, pairs a heavy
    expert with a light one per core (greedy balance), 2 experts per core.
  * Device (Bass/Tile, per core): grouped GEMM  h = x @ W1  -> swiglu ->
    y = g @ W2, weighted by combine gates.  Matmuls in bf16 with fp32
    accumulate (bf16 keeps LDWEIGHTS off the critical path).  GEMM1
    computes hT [feat, tok] so GEMM2 needs no on-chip transpose.  Token
    tiles are sized to the actual routed token counts (rounded up to 128)
    instead of the worst-case capacity; the program is cached per
    (mt0, mt1) m-tile profile.
  * Host: scatter weighted expert outputs back to token order; dense
    fallback FFN applied only to dropped tokens (none at typical loads).
"""

import os
import sys

import numpy as np


def _ensure_concourse():
    try:
        import concourse.bass  # noqa: F401
    except Exception:
        for p in ("/opt/trn_rl_repo", "/root/.axon_site/_ro/trn_rl_repo"):
            if os.path.isdir(p) and p not in sys.path:
                sys.path.insert(0, p)
        import concourse.bass  # noqa: F401


# Problem constants (hardcoded per the task contract).
B, S, D, H, E = 4, 2048, 768, 3072, 16
N = B * S
C = 640  # capacity per expert (ceil(1.25 * N / E))
FALLBACK_W = 1.0
NCORES = 8
EL = E // NCORES  # experts per core = 2
KD = D // 128  # 6 k-tiles for GEMM1 contraction
FB = (2 * H) // 128  # 48 feature blocks of GEMM1 output
FP = FB // 2  # 24 swiglu pairs == k-tiles of GEMM2 contraction
KH = H // 128  # 24
MT = C // 128  # max token m-tiles per expert
DH = 384  # output d half-tile (2 x 384 = 768)

_NC_CACHE = {}  # (mt0, mt1) -> compiled Bass program
_WCACHE = {}  # weight reorder cache
LAST = None  # BassKernelResults of the most recent run (for profiling)


def _tok_tiles(pad):
    """Split a padded token count into moving-operand tiles (<=512)."""
    out, off = [], 0
    while pad - off > 512:
        out.append((off, 512))
        off += 512
    out.append((off, pad - off))
    return out


def _build_nc(mts):
    """Per-core Bass program: 2 expert slots with mts[s] token m-tiles each."""
    import concourse.bacc as bacc
    import concourse.mybir as mybir
    import concourse.tile as tile
    from contextlib import ExitStack

    f32 = mybir.dt.float32
    bf16 = mybir.dt.bfloat16
    AF = mybir.ActivationFunctionType
    ALU = mybir.AluOpType

    pads = [m * 128 for m in mts]
    tot = sum(pads)

    nc = bacc.Bacc("TRN2", target_bir_lowering=False)
    # Host-side layouts are pre-tiled so every DMA is 2D [128, contiguous].
    xt = nc.dram_tensor("xt", [128, KD * tot], bf16, kind="ExternalInput")
    w1r = nc.dram_tensor("w1r", [EL, FP, 128, 2 * KD * 128], bf16, kind="ExternalInput")
    w2t = nc.dram_tensor("w2t", [EL, 128, KH * D], bf16, kind="ExternalInput")
    b1t = nc.dram_tensor("b1t", [EL, 128, FB], f32, kind="ExternalInput")
    wce = nc.dram_tensor("wce", [EL, 128, MT], f32, kind="ExternalInput")
    y = nc.dram_tensor("y", [tot, D], f32, kind="ExternalOutput")

    with tile.TileContext(nc) as tc, ExitStack() as ctx:
        xp = ctx.enter_context(tc.tile_pool(name="xp", bufs=2))
        w2p = ctx.enter_context(tc.tile_pool(name="w2p", bufs=2))
        gp = ctx.enter_context(tc.tile_pool(name="gp", bufs=2))
        w1p = ctx.enter_context(tc.tile_pool(name="w1p", bufs=6))
        sap = ctx.enter_context(tc.tile_pool(name="sap", bufs=3))
        cst = ctx.enter_context(tc.tile_pool(name="cst", bufs=2))
        yp = ctx.enter_context(tc.tile_pool(name="yp", bufs=4))
        p1 = ctx.enter_context(tc.tile_pool(name="p1", bufs=3, space="PSUM"))
        p2 = ctx.enter_context(tc.tile_pool(name="p2", bufs=2, space="PSUM"))

        for e in range(EL):
            pad = pads[e]
            xoff = KD * pads[0] if e else 0
            tiles = _tok_tiles(pad)
            xsb = xp.tile([128, KD * pad], bf16, tag="x")
            # per-k chunks so the first matmul doesn't wait for the full load
            for k in range(KD):
                nc.gpsimd.dma_start(
                    xsb[:, k * pad : (k + 1) * pad],
                    xt[:, xoff + k * pad : xoff + (k + 1) * pad],
                )
            b1sb = cst.tile([128, FB], f32, tag="b1")
            nc.gpsimd.dma_start(b1sb[:], b1t[e, :, :])
            wcsb = cst.tile([128, MT], f32, tag="wc")
            nc.gpsimd.dma_start(wcsb[:], wce[e, :, :])

            gt = gp.tile([128, KH * pad], bf16, tag="g")

            # GEMM1 + swiglu: hT tiles [feat 128, tok <=512]
            for fp in range(FP):
                w1t = w1p.tile([128, 2 * KD * 128], bf16, tag="w1")
                nc.sync.dma_start(w1t[:], w1r[e, fp, :, :])
                w1a = w1t[:, : KD * 128]
                w1b = w1t[:, KD * 128 :]
                for toff, tn in tiles:
                    pa = p1.tile([128, tn], f32, tag="pa")
                    pb = p1.tile([128, tn], f32, tag="pb")
                    for k in range(KD):
                        nc.tensor.matmul(
                            pa[:],
                            lhsT=w1a[:, k * 128 : (k + 1) * 128],
                            rhs=xsb[:, k * pad + toff : k * pad + toff + tn],
                            start=(k == 0),
                            stop=(k == KD - 1),
                        )
                    for k in range(KD):
                        nc.tensor.matmul(
                            pb[:],
                            lhsT=w1b[:, k * 128 : (k + 1) * 128],
                            rhs=xsb[:, k * pad + toff : k * pad + toff + tn],
                            start=(k == 0),
                            stop=(k == KD - 1),
                        )
                    sa = sap.tile([128, tn], f32, tag="sa")
                    # silu(a + b1_a)
                    nc.scalar.activation(
                        sa[:], pa[:], AF.Silu, bias=b1sb[:, fp : fp + 1], scale=1.0
                    )
                    # g = (b + b1_b) * silu(...)
                    nc.vector.scalar_tensor_tensor(
                        out=gt[:, fp * pad + toff : fp * pad + toff + tn],
                        in0=pb[:],
                        scalar=b1sb[:, FP + fp : FP + fp + 1],
                        in1=sa[:],
                        op0=ALU.add,
                        op1=ALU.mult,
                    )

            # GEMM2: y[tok 128, d 384] = sum_k g[tok, h_k] @ W2[h_k, d]
            w2sb = w2p.tile([128, KH * D], bf16, tag="w2")
            nc.gpsimd.dma_start(w2sb[:], w2t[e, :, :])
            yoff = pads[0] if e else 0
            for m in range(mts[e]):
                for dh in range(2):
                    pt = p2.tile([128, DH], f32, tag="p2")
                    for k in range(KH):
                        nc.tensor.matmul(
                            pt[:],
                            lhsT=gt[:, k * pad + m * 128 : k * pad + m * 128 + 128],
                            rhs=w2sb[:, k * D + dh * DH : k * D + (dh + 1) * DH],
                            start=(k == 0),
                            stop=(k == KH - 1),
                        )
                    ysb = yp.tile([128, DH], f32, tag="y")
                    # weighted combine: y *= gate (per-token scalar); b2 is
                    # handled host-side (it is all zeros for this problem).
                    nc.scalar.activation(
                        ysb[:], pt[:], AF.Copy, bias=0.0, scale=wcsb[:, m : m + 1]
                    )
                    nc.gpsimd.dma_start(
                        y[
                            yoff + m * 128 : yoff + (m + 1) * 128,
                            dh * DH : (dh + 1) * DH,
                        ],
                        ysb[:],
                    )
    nc.compile()
    return nc


def _get_nc(mts):
    nc = _NC_CACHE.get(mts)
    if nc is None:
        nc = _NC_CACHE[mts] = _build_nc(mts)
    return nc


def _reorder_weights(W1, W2, b1):
    key = (W1.__array_interface__["data"][0], W2.__array_interface__["data"][0])
    hit = _WCACHE.get(key)
    if hit is not None:
        return hit
    import ml_dtypes

    W1 = np.ascontiguousarray(W1, dtype=np.float32)
    W2 = np.ascontiguousarray(W2, dtype=np.float32)
    b1 = np.ascontiguousarray(b1, dtype=np.float32)
    # W1 [E, D, 2H] -> [E, FB, 128p(d within k), KD*128(f)]
    w1f = (
        W1.reshape(E, KD, 128, FB, 128)
        .transpose(0, 3, 2, 1, 4)
        .reshape(E, FB, 128, KD * 128)
        .astype(ml_dtypes.bfloat16)
    )
    # combine swiglu pair (fp, fp+FP) into one contiguous block per DMA
    w1r = np.ascontiguousarray(np.concatenate([w1f[:, :FP], w1f[:, FP:]], axis=-1))
    # W2 [E, H, D] -> [E, 128p(h within k), KH*D]
    w2t = np.ascontiguousarray(
        W2.reshape(E, KH, 128, D)
        .transpose(0, 2, 1, 3)
        .reshape(E, 128, KH * D)
        .astype(ml_dtypes.bfloat16)
    )
    # b1 [E, 2H] -> [E, 128, FB]
    b1t = np.ascontiguousarray(b1.reshape(E, FB, 128).transpose(0, 2, 1))
    out = (w1r, w2t, b1t)
    _WCACHE.clear()
    _WCACHE[key] = out
    return out


def _route(x_flat, Wr):
    logits = x_flat @ np.ascontiguousarray(Wr, dtype=np.float32)  # [N, E]
    lmax = logits.max(axis=-1, keepdims=True)
    p = np.exp(logits - lmax)
    gates = p / p.sum(axis=-1, keepdims=True)
    expert = np.argmax(gates, axis=-1)
    # slot = occurrence index of each token within its expert's queue
    order = np.argsort(expert, kind="stable")
    sorted_e = expert[order]
    starts = np.searchsorted(sorted_e, np.arange(E))
    within = np.arange(N) - starts[sorted_e]
    slot = np.empty(N, np.int64)
    slot[order] = within
    kept = slot < C
    top_idx = np.zeros((C, E), np.int32)
    valid = np.zeros((C, E), np.float32)
    tok = np.arange(N, dtype=np.int32)
    top_idx[slot[kept], expert[kept]] = tok[kept]
    valid[slot[kept], expert[kept]] = 1.0
    w_ce = gates[top_idx, np.arange(E)[None, :]].astype(np.float32) * valid  # [C, E]
    n_kept = np.minimum(np.bincount(expert, minlength=E), C)  # [E]
    return gates, expert, kept, top_idx, valid, w_ce, n_kept


def kernel(x, Wr, W1, b1, W2, b2, W1f, b1f, W2f, b2f, _trace=False):
    global LAST
    _ensure_concourse()
    import ml_dtypes
    from concourse.bass_utils import run_bass_kernel_spmd

    x_flat = np.ascontiguousarray(np.asarray(x).reshape(N, D), dtype=np.float32)
    gates, expert, kept, top_idx, valid, w_ce, n_kept = _route(x_flat, np.asarray(Wr))
    w1r, w2t, b1t = _reorder_weights(np.asarray(W1), np.asarray(W2), np.asarray(b1))

    # Pair heavy experts with light ones (greedy balance); slot 0 = heavy.
    order = np.argsort(-n_kept, kind="stable")
    assign = [(int(order[i]), int(order[E - 1 - i])) for i in range(NCORES)]
    mt_of = [max(1, int(-(-n // 128))) for n in n_kept]
    mts = (
        max(mt_of[a] for a, _ in assign),
        max(mt_of[b] for _, b in assign),
    )
    pads = [m * 128 for m in mts]

    nc = _get_nc(mts)
    in_maps = []
    for c in range(NCORES):
        exps = assign[c]
        # gather + transpose tokens for each slot: [128, KD * pad]
        xparts = []
        for s, e in enumerate(exps):
            ids = top_idx[: n_kept[e], e]
            xg = np.zeros((pads[s], D), np.float32)
            xg[: len(ids)] = x_flat[ids]
            xparts.append(
                xg.reshape(pads[s], KD, 128)
                .transpose(2, 1, 0)
                .reshape(128, KD * pads[s])
            )
        xt_c = np.ascontiguousarray(
            np.concatenate(xparts, axis=1), dtype=ml_dtypes.bfloat16
        )
        wct = np.zeros((EL, 128, MT), np.float32)
        for s, e in enumerate(exps):
            w = np.zeros(pads[s], np.float32)
            w[: n_kept[e]] = w_ce[: n_kept[e], e]
            wct[s, :, : mts[s]] = w.reshape(mts[s], 128).T
        el = list(exps)
        in_maps.append(
            {
                "xt": xt_c,
                "w1r": np.ascontiguousarray(w1r[el]),
                "w2t": np.ascontiguousarray(w2t[el]),
                "b1t": np.ascontiguousarray(b1t[el]),
                "wce": wct,
            }
        )
    res = run_bass_kernel_spmd(nc, in_maps, list(range(NCORES)), trace=_trace)
    LAST = res

    # Combine: scatter weighted expert outputs back to token order.
    y_flat = np.zeros((N, D), np.float32)
    b2 = np.asarray(b2)
    add_b2 = bool(np.any(b2))
    for c in range(NCORES):
        yc = res.results[c]["y"]
        for s, e in enumerate(assign[c]):
            n = int(n_kept[e])
            ids = top_idx[:n, e]
            off = pads[0] if s else 0
            y_flat[ids] = yc[off : off + n]
            if add_b2:
                y_flat[ids] += w_ce[:n, e][:, None] * b2[e]

    # Dense fallback for fully-dropped tokens (rare; none at typical loads).
    dropped = ~kept
    if np.any(dropped):
        xd = x_flat[dropped]
        hf = xd @ np.asarray(W1f) + np.asarray(b1f)
        gf = (hf[:, :H] / (1.0 + np.exp(-hf[:, :H]))) * hf[:, H:]
        y_flat[dropped] += FALLBACK_W * (gf @ np.asarray(W2f) + np.asarray(b2f))

    return y_flat.reshape(B, S, D)
